# revision 8
# baseline (speedup 1.0000x reference)
"""BinaryLinear on 8 trn2 NeuronCores: y = x @ sign(W)^T + bias.

x: (8192, 4096) f32, W: (4096, 4096) f32, bias: (4096,) f32 -> y: (8192, 4096) f32.

Strategy
--------
Data-parallel: shard x rows 8 x 1024 across cores; every core holds the full
binarized weight. No collectives; host concatenates the output shards.

Per-core kernel uses fp8e4 (e4m3) matmuls in DoubleRow perf mode: one
instruction contracts 256 "virtual rows" (2 fp8 rows per partition) in
~218 ns sustained at N=512 -- 2x the f32r/bf16 rate (measured; LDWEIGHTS
hides under the previous matmul via the PE reorder window).

Mixed-precision row scheme: sign(W) is exact in fp8; x is not (e4m3 RTN of
N(0,1) data has 2.65% RMS error, above the 2e-2 gate). So each of the 4096
k-indices ships either
  - two virtual rows: hi = e4m3(x_k), lo = e4m3(x_k - hi)  (error ~7e-4), or
  - one virtual row:  hi only                              (error 2.65%),
with the sign row duplicated for two-level ks. Two-level set = the n_two
columns with the LARGEST per-core residual energy sum_b (x_bk - hi_bk)^2
(weights then differ per core; x is iid so the gain is small but free).
Measured end-to-end rel error (deterministic, fixed-seed inputs, HW matches
the numpy sim to 1e-6):
  J=23 -> 1.934e-2, J=24 -> 1.817e-2; gate is 2e-2.  J = R/256 where
R = 4096 + n_two total virtual rows; J = DR instructions per output tile.

Everything else follows the f32r baseline: y produced transposed so bias
rides the PSUM partition axis (one tensor_scalar_add fuses bias + PSUM
eviction), j-outermost ramp over the first 4 o-panels while x loads, PSUM
banks rotate through full accumulation groups, sign panels on the GpSimd
DMA queue (2 panels prefetched ahead) so they never serialize ahead of the
x chunks on the Sync queue.
"""

import numpy as np
import ml_dtypes

import concourse.bass as bass  # noqa: F401  (registers engine types)
import concourse.tile as tile
from concourse import bacc, mybir
from concourse.bass_utils import run_bass_kernel_spmd

NCORES = 8
M_FULL, K, O = 8192, 4096, 4096
M = M_FULL // NCORES          # 1024 rows of x per core
P = 128                       # partition width
OT = O // P                   # 32 o-tiles
NM = 512                      # moving free dim per matmul (output columns)
MB = M // NM                  # 2 m-blocks
J = 23                        # DR steps per group; R = J*256 virtual rows
R = J * 256
N_TWO = R - K                 # number of two-level k-indices
RAMP_OT = 4                   # o-tiles interleaved j-outer during the x load

_F8 = mybir.dt.float8e4
_F32 = mybir.dt.float32
_DR = mybir.MatmulPerfMode.DoubleRow
_NPF8 = ml_dtypes.float8_e4m3

_COMPILED = None


def _build():
    nc = bacc.Bacc("TRN2", target_bir_lowering=False, debug=False)
    xt_ap = nc.dram_tensor("xt", [P, J, 2, M], _F8, kind="ExternalInput").ap()
    st_ap = nc.dram_tensor("st", [OT, P, J, 2, P], _F8, kind="ExternalInput").ap()
    b_ap = nc.dram_tensor("biasc", [P, OT], _F32, kind="ExternalInput").ap()
    yt_ap = nc.dram_tensor("yt", [O, M], _F32, kind="ExternalOutput").ap()
    yt_r = yt_ap.rearrange("(ot p) m -> ot p m", p=P)

    from contextlib import ExitStack

    with tile.TileContext(nc) as tc:
        with ExitStack() as ctx:
            xpool = ctx.enter_context(tc.tile_pool(name="x", bufs=J))
            spool = ctx.enter_context(tc.tile_pool(name="s", bufs=6))
            bpool = ctx.enter_context(tc.tile_pool(name="b", bufs=1))
            ypool = ctx.enter_context(tc.tile_pool(name="y", bufs=3))
            psum = ctx.enter_context(tc.tile_pool(name="ps", bufs=8, space="PSUM"))

            # Bias rides the GpSimd trigger queue: the Sync engine's serial
            # ~675 ns/trigger issue pipe is reserved for the x chunks.
            b_sb = bpool.tile([P, OT], _F32)
            nc.gpsimd.dma_start(b_sb[:], b_ap[:])

            # Prewarm the PE so HAM un-throttles (1.2 -> 2.4 GHz) before the
            # real matmuls: ~5 us of dummy work on a scratch tile, discarded.
            scratch = bpool.tile([P, 256], _F32)
            nc.vector.memset(scratch[:], 1.0)
            warm_ps = psum.tile([P, 256], _F32, name="ps_warm", tag="ps")
            for _ in range(12):
                nc.tensor.matmul(
                    warm_ps[:], scratch[:, :P], scratch[:], start=True, stop=True
                )

            S_PC = 8  # j per sign-panel DMA piece (finer deps -> earlier mms)

            def load_panel(ot):
                s_sb = spool.tile([P, J, 2, P], _F8, name=f"s{ot}", tag="s")
                for pc in range(0, J, S_PC):
                    pe = min(pc + S_PC, J)
                    nc.gpsimd.dma_start(
                        s_sb[:, pc:pe, :, :], st_ap[ot][:, pc:pe, :, :]
                    )
                return s_sb

            # Whole packed-x shard resident in SBUF (5.9 MB), one tile per
            # DR step so matmuls only depend on the chunk they read. One
            # whole-chunk DMA per step: each DMA_DIRECT2D trigger costs
            # ~675 ns on the Sync engine's issue pipe, so splitting chunks
            # for finer arrival granularity backfires -- 43 split triggers
            # took ~29 us to issue and starved the ramp, while transfer
            # itself sustains a chunk every ~1.3 us vs the PE's 1.7 us
            # per-chunk ramp consumption.
            x_tiles = []
            for j in range(J):
                xt = xpool.tile([P, 2, M], _F8, name=f"x{j}", tag="x")
                nc.sync.dma_start(xt[:], xt_ap[:, j, :, :])
                x_tiles.append(xt)

            # Ramp sign panels stream on the GpSimd queue in parallel with
            # the x load, pieces interleaved across panels so every panel's
            # first j-steps are ready as soon as possible.
            s_tiles = {
                ot: spool.tile([P, J, 2, P], _F8, name=f"s{ot}", tag="s")
                for ot in range(RAMP_OT)
            }
            for pc in range(0, J, S_PC):
                pe = min(pc + S_PC, J)
                for ot in range(RAMP_OT):
                    nc.gpsimd.dma_start(
                        s_tiles[ot][:, pc:pe, :, :], st_ap[ot][:, pc:pe, :, :]
                    )

            def drain(ps, ot, mb):
                y_sb = ypool.tile([P, NM], _F32, name=f"y{ot}_{mb}", tag="y")
                nc.vector.tensor_scalar_add(y_sb[:], ps[:], b_sb[:, ot:ot + 1])
                nc.sync.dma_start(yt_r[ot][:, mb * NM:(mb + 1) * NM], y_sb[:])

            # Ramp: j-outer over the first RAMP_OT panels' groups, so the PE
            # issues work for x chunk j as soon as that chunk's DMA lands
            # instead of stalling in-order behind the full x load.
            # ot-major so each (ot, j) weight tile is loaded once and used
            # for both mb matmuls back-to-back: 1 LDWEIGHTS per 2 matmuls
            # keeps the ~200 ns DR weight load off the critical path.
            groups = [(ot, mb) for ot in range(RAMP_OT) for mb in range(MB)]
            ramp_ps = {
                g: psum.tile([P, NM], _F32, name=f"ps_r{g[0]}_{g[1]}", tag="ps")
                for g in groups
            }
            for j in range(J):
                for (ot, mb) in groups:
                    nc.tensor.matmul(
                        ramp_ps[(ot, mb)][:],
                        s_tiles[ot][:, j, :, :],
                        x_tiles[j][:, :, mb * NM:(mb + 1) * NM],
                        start=(j == 0),
                        stop=(j == J - 1),
                        perf_mode=_DR,
                    )
            # Prefetch the next two steady panels before the ramp drains so
            # the PE never waits on the GpSimd DMA queue at the handoff.
            pending = {
                RAMP_OT: load_panel(RAMP_OT),
                RAMP_OT + 1: load_panel(RAMP_OT + 1),
            }
            for (ot, mb) in groups:
                drain(ramp_ps[(ot, mb)], ot, mb)

            # Steady state: j-inner accumulation, one PSUM bank per group,
            # panel DMA pipelined 2 o-tiles ahead of use.
            for ot in range(RAMP_OT, OT):
                s_sb = pending.pop(ot)
                if ot + 2 < OT:
                    pending[ot + 2] = load_panel(ot + 2)
                # j outer / mb inner: both m-blocks accumulate in parallel
                # banks, so each weight tile serves 2 consecutive matmuls.
                pss = [
                    psum.tile([P, NM], _F32, name=f"ps_{ot}_{mb}", tag="ps")
                    for mb in range(MB)
                ]
                for j in range(J):
                    for mb in range(MB):
                        nc.tensor.matmul(
                            pss[mb][:],
                            s_sb[:, j, :, :],
                            x_tiles[j][:, :, mb * NM:(mb + 1) * NM],
                            start=(j == 0),
                            stop=(j == J - 1),
                            perf_mode=_DR,
                        )
                for mb in range(MB):
                    drain(pss[mb], ot, mb)

    nc.compile()
    return nc


def _get_compiled():
    global _COMPILED
    if _COMPILED is None:
        _COMPILED = _build()
    return _COMPILED


def _pack_inputs(x, weight, bias):
    x = np.ascontiguousarray(x, dtype=np.float32)
    s = np.sign(weight).astype(np.float32)          # (O, K)
    st_base = s.T                                   # (K, O)

    biasc = np.ascontiguousarray(
        np.asarray(bias, dtype=np.float32).reshape(OT, P).T
    )

    in_maps = []
    for c in range(NCORES):
        xs = x[c * M:(c + 1) * M].T                  # (K, M)
        hi = xs.astype(_NPF8)                        # (K, M) fp8
        resid = xs - hi.astype(np.float32)
        # Two-level set: columns with the largest residual energy.
        T = np.argsort(-(resid ** 2).sum(axis=1))[:N_TWO]
        T.sort()
        lo = resid[T].astype(_NPF8)
        d = np.concatenate([hi, lo], axis=0)         # (R, M) fp8
        xt = np.ascontiguousarray(
            d.reshape(J, 2, P, M).transpose(2, 0, 1, 3)
        )
        # Weights: virtual row g carries weight vector s[:, k(g)]
        row_k = np.concatenate([np.arange(K), T])
        wfull = st_base[row_k].astype(_NPF8)         # (R, O)
        st = np.ascontiguousarray(
            wfull.reshape(J, 2, P, OT, P).transpose(3, 2, 0, 1, 4)
        )
        in_maps.append({"xt": xt, "st": st, "biasc": biasc})
    return in_maps


def _run(x, weight, bias, trace=False):
    nc = _get_compiled()
    in_maps = _pack_inputs(x, weight, bias)
    res = run_bass_kernel_spmd(nc, in_maps, list(range(NCORES)), trace=trace)
    y = np.empty((M_FULL, O), dtype=np.float32)
    for c in range(NCORES):
        y[c * M:(c + 1) * M] = res.results[c]["yt"].T
    return y, res


def kernel(x, weight, bias):
    y, _ = _run(x, weight, bias, trace=False)
    return y
